# revision 4
# baseline (speedup 1.0000x reference)
"""Decoder-only attention block (QKV proj + MHA + out proj) on 8 TRN2 cores.

Sharding: core c -> (batch b = c//4, head-group g = c%4). Tensor-parallel over
heads (4 of 16 heads per core), data-parallel over batch (2). Each core
computes a partial c_proj over its 512 input features; host reduces the 4
partials per batch and adds biases.

Schedule: V projection first (it needs only ~9 MB of input, so it hides the
bulk DMA stream), then Q/K projection with 8 concurrent PSUM chains, then a
pair-granularity weave of score matmuls with AV / c_proj / leftover-V filler
so the in-order Tensor queue never blocks on the scalar-engine exp pipeline.

Self-contained: hardcodes B=2, S=2048, D=2048, H=16.
"""

import os

import numpy as np

NPF16 = np.float16

import concourse.bass as bass
import concourse.bacc as bacc
import concourse.tile as tile
from concourse import mybir
import concourse.bass_utils as bass_utils
import concourse.bass_isa as bass_isa
from concourse.bass_interp import get_hw_module

B, S, D = 2, 2048, 2048
H, DH = 16, 128
N_CORES = 8
HL = H // 4            # 4 heads per core
FL = HL * DH           # 512 local features per core
KT = D // 128          # 16 contraction tiles
TT = S // 128          # 16 token tiles
QB = S // 512          # 4 token blocks
SCALE = 1.0 / float(np.sqrt(DH))

F16 = mybir.dt.float16
F32 = mybir.dt.float32

# Stash of the last BassKernelResults (for the local test harness only).
LAST_RESULTS = None
_PROG_CACHE = {}


def _build_program(use_mask):

    nc = bacc.Bacc("TRN2", target_bir_lowering=False, debug=False,
                   num_devices=N_CORES)

    # kt-major x for QK (moving operand), t-major copy for V (stationary).
    xt_d = nc.dram_tensor("xt", [128, KT * S], F16, kind="ExternalInput")
    xv_d = nc.dram_tensor("xv", [128, TT * S], F16, kind="ExternalInput")
    # head-pair-major QK weights: [row, h*4096 + kt*256 + half*128 + col]
    wqk_d = nc.dram_tensor("wqk", [128, HL * 4096], F16, kind="ExternalInput")
    wv_d = nc.dram_tensor("wv", [128, KT * FL], F16, kind="ExternalInput")
    wp_d = nc.dram_tensor("wp", [128, HL * D], F16, kind="ExternalInput")
    bqk_d = nc.dram_tensor("bqk", [128, 8], F32, kind="ExternalInput")
    kb_d = nc.dram_tensor("kb", [128, KT], F32, kind="ExternalInput")
    out_d = nc.dram_tensor("out", [S, D], F16, kind="ExternalOutput")

    xt_ap, xv_ap, wqk_ap = xt_d.ap(), xv_d.ap(), wqk_d.ap()
    wv_ap, wp_ap = wv_d.ap(), wp_d.ap()
    bqk_ap, kb_ap, out_ap = bqk_d.ap(), kb_d.ap(), out_d.ap()

    with tile.TileContext(nc) as tc, tc.tile_pool(name="pers", bufs=1) as pers:
        # ---- persistent tiles (live across phases) ----
        qt = [pers.tile([128, S], F16, tag=f"qt{h}", name=f"qt{h}") for h in range(HL)]
        ktt = [pers.tile([128, S], F16, tag=f"kt{h}", name=f"ktt{h}") for h in range(HL)]
        ot = [pers.tile([128, S], F16, tag=f"ot{h}", name=f"ot{h}") for h in range(HL)]
        vaug = [[pers.tile([128, DH], F16, tag=f"v{t}_{h}", name=f"v{t}_{h}")
                 for h in range(HL)] for t in range(TT)]
        wp_sb = pers.tile([128, HL * D], F16, tag="wp", name="wp_sb")
        wv_sb = pers.tile([128, KT * FL], F16, tag="wv", name="wv_sb")
        bqk_sb = pers.tile([128, 8], F32, tag="bqk", name="bqk_sb")
        kb_sb = pers.tile([128, KT], F32, tag="kb", name="kb_sb")
        warm = pers.tile([128, 512], F16, tag="warm", name="warm")

        # small side transfers ride the SWDGE queue; bulk uses the two HWDGE
        # rings (sync + scalar).
        nc.gpsimd.dma_start(bqk_sb[:], bqk_ap[:])
        nc.gpsimd.dma_start(kb_sb[:], kb_ap[:])
        nc.vector.memset(warm[:], 0.0)

        # rolling t-major x buffers for V (persistent so DMA writes never
        # alias phase-local pools).
        def xv_dma(t):
            xvt = pers.tile([128, S], F16, tag="xv", bufs=2, name="xv")
            nc.sync.dma_start(xvt[:], xv_ap[:, t * S:(t + 1) * S])
            return xvt

        def v_chunks(t, pool, tag, bufs):
            box = [None]

            def mk(c):
                def go():
                    if c == 0:
                        box[0] = (pool.tile([128, FL], F32, tag=tag,
                                            bufs=bufs, name=tag), xv_dma(t))
                    psv, xvt = box[0]
                    for kt in range(4 * c, 4 * c + 4):
                        nc.tensor.matmul(
                            psv[:],
                            xvt[:, kt * 128:(kt + 1) * 128],
                            wv_sb[:, kt * FL:(kt + 1) * FL],
                            start=(kt == 0), stop=(kt == KT - 1),
                        )
                    if c == 3:
                        for h in range(HL):
                            nc.vector.tensor_copy(vaug[t][h][:],
                                                  psv[:, h * 128:(h + 1) * 128])
                return go

            return [mk(c) for c in range(4)]

        # ---- phase 0: V tiles 0-9 (covers the input DMA stream) ----
        with tc.tile_pool(name="psv0", bufs=1, space="PSUM") as psv0:
            nc.sync.dma_start(wv_sb[:, 0:8 * FL], wv_ap[:, 0:8 * FL])
            # PE warm-up: HAM needs ~3.4us of sustained activity to lift the
            # clock gate; burn the initial DMA wait on dummy matmuls.
            wps = psv0.tile([128, FL], F32, tag="psv", bufs=2, name="wps")
            for _ in range(12):
                nc.tensor.matmul(wps[:], warm[:, 0:128], warm[:],
                                 start=True, stop=True, skip_group_check=True)
            u0 = v_chunks(0, psv0, "psv", 2)
            u0[0]()
            nc.sync.dma_start(wv_sb[:, 8 * FL:KT * FL],
                              wv_ap[:, 8 * FL:KT * FL])
            for u in u0[1:]:
                u()
            for t in range(1, 10):
                for u in v_chunks(t, psv0, "psv", 2):
                    u()

        # ---- phase 1: Q/K projection, 8 concurrent PSUM chains ----
        with (
            tc.tile_pool(name="pxt", bufs=1) as pxt,
            tc.tile_pool(name="pwqk", bufs=1) as pwqk,
            tc.tile_pool(name="psqk", bufs=1, space="PSUM") as psqk,
        ):
            xt_sb = pxt.tile([128, KT * S], F16, tag="xt", name="xt_sb")
            wqk_sb = pwqk.tile([128, HL * 4096], F16, tag="wqk", name="wqk_sb")

            nc.sync.dma_start(wqk_sb[:, 0:4096], wqk_ap[:, 0:4096])
            for k0 in range(0, KT, 4):
                nc.scalar.dma_start(xt_sb[:, k0 * S:(k0 + 4) * S],
                                    xt_ap[:, k0 * S:(k0 + 4) * S])
            for p in range(1, HL):
                nc.scalar.dma_start(wqk_sb[:, p * 4096:(p + 1) * 4096],
                                    wqk_ap[:, p * 4096:(p + 1) * 4096])
            nc.scalar.dma_start(wp_sb[:], wp_ap[:])

            for h in range(HL):
                ps = [psqk.tile([128, 512], F32, tag=f"psqk{i}", bufs=1,
                                name=f"psqk{i}") for i in range(8)]
                for kt in range(KT):
                    for half in range(2):
                        base = h * 4096 + kt * 256 + half * 128
                        wsl = wqk_sb[:, base:base + 128]
                        for tb in range(4):
                            nc.tensor.matmul(
                                ps[half * 4 + tb][:],
                                wsl,
                                xt_sb[:, kt * S + tb * 512:kt * S + (tb + 1) * 512],
                                start=(kt == 0), stop=(kt == KT - 1),
                                skip_group_check=True,
                            )
                # evac split across scalar+vector for faster bank turnover
                for i in range(8):
                    half, tb = divmod(i, 4)
                    dest = (qt if half == 0 else ktt)[h]
                    col = h if half == 0 else 4 + h
                    dsl = dest[:, tb * 512:(tb + 1) * 512]
                    if i % 2 == 0:
                        nc.scalar.add(dsl, ps[i][:], bqk_sb[:, col:col + 1])
                    else:
                        nc.vector.tensor_scalar_add(dsl, ps[i][:],
                                                    bqk_sb[:, col:col + 1])

        # ---- phase 2: attention + c_proj, pair-granularity weave ----
        with (
            tc.tile_pool(name="p2", bufs=1) as p2,
            tc.tile_pool(name="ps2a", bufs=1, space="PSUM") as ps2a,
        ):
            e_store = {}
            rcp_store = {}
            cnt = [0]

            def s_pairs(qb, h):
                es = []
                e_store[(qb, h)] = es

                def mk(p):
                    def go():
                        pss = ps2a.tile([128, 1024], F32, tag="pss", bufs=2,
                                        name="pss")
                        for half in range(2):
                            kt = 2 * p + half
                            nc.tensor.matmul(
                                pss[:, half * 512:(half + 1) * 512],
                                ktt[h][:, kt * 128:(kt + 1) * 128],
                                qt[h][:, qb * 512:(qb + 1) * 512],
                                start=True, stop=True,
                            )
                        e = p2.tile([128, 1024], F16, tag=f"e{p}", bufs=3,
                                    name=f"e{p}")
                        nc.scalar.activation(
                            e[:], pss[:], mybir.ActivationFunctionType.Exp,
                            scale=SCALE,
                        )
                        if use_mask:
                            for half in range(2):
                                kt = 2 * p + half
                                sl = e[:, half * 512:(half + 1) * 512]
                                nc.vector.tensor_scalar_mul(
                                    sl, sl, kb_sb[:, kt:kt + 1])
                        es.append(e)
                    return go

                def tail():
                    # denominator: first add level on GpSimd (DVE is the
                    # scarcer engine), rest on DVE, partition all-reduce on
                    # GpSimd, reciprocal on DVE.
                    l1s = []
                    for i in range(4):
                        l1 = p2.tile([128, 1024], F16, tag=f"l1_{i % 2}",
                                     bufs=2, name=f"l1_{i % 2}")
                        nc.gpsimd.tensor_add(l1[:], es[2 * i][:],
                                             es[2 * i + 1][:])
                        l1s.append(l1)
                    l2s = []
                    for j in range(2):
                        l2 = p2.tile([128, 1024], F16, tag=f"l2_{j}", bufs=2,
                                     name=f"l2_{j}")
                        nc.vector.tensor_add(l2[:], l1s[2 * j][:],
                                             l1s[2 * j + 1][:])
                        l2s.append(l2)
                    l3 = p2.tile([128, 1024], F16, tag="l3", bufs=2, name="l3")
                    nc.vector.tensor_add(l3[:], l2s[0][:], l2s[1][:])
                    dn = p2.tile([128, 512], F32, tag="dn", bufs=2, name="dn")
                    nc.vector.tensor_add(dn[:], l3[:, 0:512], l3[:, 512:1024])
                    dnr = p2.tile([128, 512], F32, tag="dnr", bufs=2,
                                  name="dnr")
                    nc.gpsimd.partition_all_reduce(
                        dnr[:], dn[:], channels=128,
                        reduce_op=bass_isa.ReduceOp.add)
                    rcp = p2.tile([128, 512], F32, tag="rcp", bufs=2,
                                  name="rcp")
                    nc.vector.reciprocal_approx_fast(rcp[:], dnr[:])
                    rcp_store[(qb, h)] = rcp

                return [mk(p) for p in range(8)], tail

            def a_chunks(qb, h):
                psot_box = [None]

                def mk(c):
                    def go():
                        if c == 0:
                            psot_box[0] = ps2a.tile([128, 512], F32,
                                                    tag="psot", bufs=2,
                                                    name="psot")
                        es = e_store[(qb, h)]
                        for kt in range(4 * c, 4 * c + 4):
                            nc.tensor.matmul(
                                psot_box[0][:],
                                vaug[kt][h][:],
                                es[kt // 2][:, (kt % 2) * 512:(kt % 2 + 1) * 512],
                                start=(kt == 0), stop=(kt == KT - 1),
                            )
                        if c == 3:
                            nc.vector.tensor_mul(
                                ot[h][:, qb * 512:(qb + 1) * 512],
                                psot_box[0][:], rcp_store.pop((qb, h))[:])
                            e_store.pop((qb, h))
                    return go

                return [mk(c) for c in range(4)]

            stage_box = [None]

            def c_units(t, pool):
                def mk(nb):
                    def go():
                        if nb == 0:
                            stage_box[0] = p2.tile([128, S], F16, tag="stage",
                                                   bufs=3, name="stage")
                        psp = pool.tile([128, 512], F32,
                                        tag=f"psp{cnt[0] % 2}", bufs=1,
                                        name=f"psp{cnt[0] % 2}")
                        cnt[0] += 1
                        for h in range(HL):
                            nc.tensor.matmul(
                                psp[:],
                                ot[h][:, t * 128:(t + 1) * 128],
                                wp_sb[:, h * D + nb * 512:h * D + (nb + 1) * 512],
                                start=(h == 0), stop=(h == HL - 1),
                                skip_group_check=True,
                            )
                        st = stage_box[0]
                        nc.vector.tensor_copy(st[:, nb * 512:(nb + 1) * 512],
                                              psp[:])
                        if nb == 3:
                            eng = nc.sync if t % 2 else nc.scalar
                            eng.dma_start(out_ap[t * 128:(t + 1) * 128, :],
                                          st[:])
                    return go

                return [mk(nb) for nb in range(4)]

            def weave(pairs_tail, units):
                pairs, tail = pairs_tail
                for i in range(8):
                    pairs[i]()
                    if i < len(units):
                        units[i]()
                for u in units[8:]:
                    u()
                tail()

            # leftover V tiles ride the psot tag (same shape/banks, first AV
            # comes after the last of these).
            weave(s_pairs(0, 0),
                  v_chunks(10, ps2a, "psot", 2) + v_chunks(11, ps2a, "psot", 2))
            weave(s_pairs(0, 1),
                  v_chunks(12, ps2a, "psot", 2) + v_chunks(13, ps2a, "psot", 2))
            weave(s_pairs(0, 2),
                  v_chunks(14, ps2a, "psot", 2) + v_chunks(15, ps2a, "psot", 2))
            weave(s_pairs(0, 3), a_chunks(0, 0) + a_chunks(0, 1))

            with tc.tile_pool(name="ps2c", bufs=1, space="PSUM") as ps2c:
                weave(s_pairs(1, 0), a_chunks(0, 2) + a_chunks(0, 3))
                # steady slots: S(qb,h) ⊗ [A(prev head), c_proj tile]
                slots = [(qb, h) for qb in range(1, QB) for h in range(HL)][1:]
                for m, (qb, h) in enumerate(slots):
                    prev = (qb, h - 1) if h > 0 else (qb - 1, 3)
                    units = a_chunks(*prev) + c_units(m, ps2c)
                    if (qb, h) == (QB - 1, HL - 1):
                        units += c_units(m + 1, ps2c)
                    weave(s_pairs(qb, h), units)
                for u in a_chunks(QB - 1, 3):
                    u()
                for t in range(12, TT):
                    for u in c_units(t, ps2c):
                        u()

    nc.compile()
    nc.m = get_hw_module(nc.m)
    return nc


def kernel(hidden_states, attention_mask, w_attn, b_attn, w_proj, b_proj):
    global LAST_RESULTS
    hidden_states = np.asarray(hidden_states, dtype=np.float32)
    attention_mask = np.asarray(attention_mask, dtype=np.float32)
    w_attn = np.asarray(w_attn, dtype=np.float32)
    b_attn = np.asarray(b_attn, dtype=np.float32)
    w_proj = np.asarray(w_proj, dtype=np.float32)
    b_proj = np.asarray(b_proj, dtype=np.float32)

    use_mask = bool((attention_mask != 1.0).any())
    key = ("prog", use_mask)
    if key not in _PROG_CACHE:
        _PROG_CACHE[key] = _build_program(use_mask)
    nc = _PROG_CACHE[key]

    in_maps = []
    for c in range(N_CORES):
        b, g = divmod(c, 4)
        X = np.ascontiguousarray(hidden_states[b].T).astype(NPF16)  # [D, S]
        xt = np.ascontiguousarray(
            X.reshape(KT, 128, S).transpose(1, 0, 2).reshape(128, KT * S))
        xv = np.ascontiguousarray(
            X.reshape(KT, 128, TT, 128).transpose(1, 2, 0, 3)
            .reshape(128, TT * S))
        wq = w_attn[:, g * FL:(g + 1) * FL]
        wk = w_attn[:, D + g * FL:D + (g + 1) * FL]
        wvl = w_attn[:, 2 * D + g * FL:2 * D + (g + 1) * FL]
        A = wq.reshape(KT, 128, HL, 128)
        Bm = wk.reshape(KT, 128, HL, 128)
        wqk = np.ascontiguousarray(
            np.stack([A, Bm], axis=3).transpose(1, 2, 0, 3, 4)
            .reshape(128, HL * 4096)).astype(NPF16)
        wv = np.ascontiguousarray(
            wvl.reshape(KT, 128, FL).transpose(1, 0, 2)
            .reshape(128, KT * FL)).astype(NPF16)
        wp = np.ascontiguousarray(
            w_proj[g * FL:(g + 1) * FL, :].reshape(HL, 128, D)
            .transpose(1, 0, 2).reshape(128, HL * D)).astype(NPF16)
        bq = b_attn[g * FL:(g + 1) * FL]
        bk = b_attn[D + g * FL:D + (g + 1) * FL]
        bqk = np.ascontiguousarray(
            np.concatenate([bq, bk]).reshape(8, 128).T).astype(np.float32)
        kb = np.ascontiguousarray(
            attention_mask[b].reshape(KT, 128).T).astype(np.float32)
        in_maps.append({
            "xt": xt,
            "xv": xv,
            "wqk": wqk,
            "wv": wv,
            "wp": wp,
            "bqk": bqk,
            "kb": kb,
        })

    if not os.environ.get("KERNEL_ALLOW_TRACE"):
        os.environ["BASS_NEVER_TRACE"] = "1"
    try:
        res = bass_utils.run_bass_kernel_spmd(nc, in_maps,
                                              list(range(N_CORES)))
    except Exception:
        # Transient NRT failures can leave the axon device wedged; reset it
        # once and retry. If the reset path is unavailable, the retry's own
        # failure propagates.
        try:
            import ctypes

            import jax

            jax.devices()
            _lib = ctypes.CDLL("/opt/axon/libaxon_pjrt.so")
            _lib.axon_reset.restype = ctypes.c_int64
            _lib.axon_reset()
        except Exception:
            pass
        res = bass_utils.run_bass_kernel_spmd(nc, in_maps,
                                              list(range(N_CORES)))
    LAST_RESULTS = res

    # host reduce: sum the 4 head-group partials per batch, add biases.
    # V-bias contribution: rows of A sum to 1, so each core's O gains b_v
    # per row; through c_proj that's a constant row b_v @ w_proj_slice.
    out = np.zeros((B, S, D), dtype=np.float32)
    for c in range(N_CORES):
        b, g = divmod(c, 4)
        out[b] += res.results[c]["out"].astype(np.float32)
    bias_row = b_proj.astype(np.float64).copy()
    for g in range(4):
        bv = b_attn[2 * D + g * FL:2 * D + (g + 1) * FL].astype(np.float64)
        bias_row += bv @ w_proj[g * FL:(g + 1) * FL, :].astype(np.float64)
    out += bias_row.astype(np.float32)[None, None, :]
    return out


# revision 6
# speedup vs baseline: 1.5584x; 1.5584x over previous
"""Decoder-only attention block (QKV proj + MHA + out proj) on 8 TRN2 cores.

Sharding: core c -> (batch b = c//4, head-group g = c%4). Tensor-parallel over
heads (4 of 16 heads per core), data-parallel over batch (2). Each core
computes a partial c_proj over its 512 input features; host reduces the 4
partials per batch and adds biases.

Schedule: V projection first (it needs only ~9 MB of input, so it hides the
bulk DMA stream), then Q/K projection with 8 concurrent PSUM chains, then a
pair-granularity weave of score matmuls with AV / c_proj / leftover-V filler
so the in-order Tensor queue never blocks on the scalar-engine exp pipeline.

Self-contained: hardcodes B=2, S=2048, D=2048, H=16.
"""

import os

import numpy as np

NPF16 = np.float16

import concourse.bass as bass
import concourse.bacc as bacc
import concourse.tile as tile
from concourse import mybir
import concourse.bass_utils as bass_utils
import concourse.bass_isa as bass_isa
from concourse.bass_interp import get_hw_module

B, S, D = 2, 2048, 2048
H, DH = 16, 128
N_CORES = 8
HL = H // 4            # 4 heads per core
FL = HL * DH           # 512 local features per core
KT = D // 128          # 16 contraction tiles
TT = S // 128          # 16 token tiles
QB = S // 512          # 4 token blocks
SCALE = 1.0 / float(np.sqrt(DH))

F16 = mybir.dt.float16
F32 = mybir.dt.float32

# Stash of the last BassKernelResults (for the local test harness only).
LAST_RESULTS = None
_PROG_CACHE = {}


def _build_program(use_mask):

    nc = bacc.Bacc("TRN2", target_bir_lowering=False, debug=False,
                   num_devices=N_CORES)

    # kt-major x for QK (moving operand), t-major copy for V (stationary).
    xt_d = nc.dram_tensor("xt", [128, KT * S], F16, kind="ExternalInput")
    xv_d = nc.dram_tensor("xv", [128, TT * S], F16, kind="ExternalInput")
    # head-pair-major QK weights: [row, h*4096 + kt*256 + half*128 + col]
    wqk_d = nc.dram_tensor("wqk", [128, HL * 4096], F16, kind="ExternalInput")
    wv_d = nc.dram_tensor("wv", [128, KT * FL], F16, kind="ExternalInput")
    wp_d = nc.dram_tensor("wp", [128, HL * D], F16, kind="ExternalInput")
    bqk_d = nc.dram_tensor("bqk", [128, 8], F32, kind="ExternalInput")
    kb_d = nc.dram_tensor("kb", [128, KT], F32, kind="ExternalInput")
    out_d = nc.dram_tensor("out", [S, D], F16, kind="ExternalOutput")

    xt_ap, xv_ap, wqk_ap = xt_d.ap(), xv_d.ap(), wqk_d.ap()
    wv_ap, wp_ap = wv_d.ap(), wp_d.ap()
    bqk_ap, kb_ap, out_ap = bqk_d.ap(), kb_d.ap(), out_d.ap()

    with tile.TileContext(nc) as tc, tc.tile_pool(name="pers", bufs=1) as pers:
        # ---- persistent tiles (live across phases) ----
        qt = [pers.tile([128, S], F16, tag=f"qt{h}", name=f"qt{h}") for h in range(HL)]
        ktt = [pers.tile([128, S], F16, tag=f"kt{h}", name=f"ktt{h}") for h in range(HL)]
        ot = [pers.tile([128, S], F16, tag=f"ot{h}", name=f"ot{h}") for h in range(HL)]
        vaug = [[pers.tile([128, DH], F16, tag=f"v{t}_{h}", name=f"v{t}_{h}")
                 for h in range(HL)] for t in range(TT)]
        wp_sb = pers.tile([128, HL * D], F16, tag="wp", name="wp_sb")
        wv_sb = pers.tile([128, KT * FL], F16, tag="wv", name="wv_sb")
        bqk_sb = pers.tile([128, 8], F32, tag="bqk", name="bqk_sb")
        kb_sb = pers.tile([128, KT], F32, tag="kb", name="kb_sb")
        warm = pers.tile([128, 512], F16, tag="warm", name="warm")

        # small side transfers ride the SWDGE queue; bulk uses the two HWDGE
        # rings (sync + scalar).
        nc.gpsimd.dma_start(bqk_sb[:], bqk_ap[:])
        nc.gpsimd.dma_start(kb_sb[:], kb_ap[:])
        nc.vector.memset(warm[:], 0.0)

        # rolling t-major x buffers for V (persistent so DMA writes never
        # alias phase-local pools).
        def xv_dma(t):
            xvt = pers.tile([128, S], F16, tag="xv", bufs=2, name="xv")
            nc.sync.dma_start(xvt[:], xv_ap[:, t * S:(t + 1) * S])
            return xvt

        def v_chunks(t, pool, tag, bufs):
            box = [None]

            def mk(c):
                def go():
                    if c == 0:
                        box[0] = (pool.tile([128, FL], F32, tag=tag,
                                            bufs=bufs, name=tag), xv_dma(t))
                    psv, xvt = box[0]
                    for kt in range(4 * c, 4 * c + 4):
                        nc.tensor.matmul(
                            psv[:],
                            xvt[:, kt * 128:(kt + 1) * 128],
                            wv_sb[:, kt * FL:(kt + 1) * FL],
                            start=(kt == 0), stop=(kt == KT - 1),
                        )
                    if c == 3:
                        for h in range(HL):
                            nc.vector.tensor_copy(vaug[t][h][:],
                                                  psv[:, h * 128:(h + 1) * 128])
                return go

            return [mk(c) for c in range(4)]

        # ---- phase 0: V tiles 0-9 (covers the input DMA stream) ----
        with tc.tile_pool(name="psv0", bufs=1, space="PSUM") as psv0:
            nc.sync.dma_start(wv_sb[:, 0:8 * FL], wv_ap[:, 0:8 * FL])
            # PE warm-up: HAM needs ~3.4us of sustained activity to lift the
            # clock gate; burn the initial DMA wait on dummy matmuls.
            wps = psv0.tile([128, FL], F32, tag="psv", bufs=2, name="wps")
            for _ in range(12):
                nc.tensor.matmul(wps[:], warm[:, 0:128], warm[:],
                                 start=True, stop=True, skip_group_check=True)
            u0 = v_chunks(0, psv0, "psv", 2)
            u0[0]()
            nc.sync.dma_start(wv_sb[:, 8 * FL:KT * FL],
                              wv_ap[:, 8 * FL:KT * FL])
            for u in u0[1:]:
                u()
            for t in range(1, 10):
                for u in v_chunks(t, psv0, "psv", 2):
                    u()

        # ---- phase 1: Q/K projection, 8 concurrent PSUM chains ----
        with (
            tc.tile_pool(name="pxt", bufs=1) as pxt,
            tc.tile_pool(name="pwqk", bufs=1) as pwqk,
            tc.tile_pool(name="psqk", bufs=1, space="PSUM") as psqk,
        ):
            xt_sb = pxt.tile([128, KT * S], F16, tag="xt", name="xt_sb")
            wqk_sb = pwqk.tile([128, HL * 4096], F16, tag="wqk", name="wqk_sb")

            nc.sync.dma_start(wqk_sb[:, 0:4096], wqk_ap[:, 0:4096])
            for k0 in range(0, KT, 4):
                nc.scalar.dma_start(xt_sb[:, k0 * S:(k0 + 4) * S],
                                    xt_ap[:, k0 * S:(k0 + 4) * S])
            for p in range(1, HL):
                nc.scalar.dma_start(wqk_sb[:, p * 4096:(p + 1) * 4096],
                                    wqk_ap[:, p * 4096:(p + 1) * 4096])
            nc.scalar.dma_start(wp_sb[:], wp_ap[:])

            for h in range(HL):
                ps = [psqk.tile([128, 512], F32, tag=f"psqk{i}", bufs=1,
                                name=f"psqk{i}") for i in range(8)]
                for kt in range(KT):
                    for half in range(2):
                        base = h * 4096 + kt * 256 + half * 128
                        wsl = wqk_sb[:, base:base + 128]
                        for tb in range(4):
                            nc.tensor.matmul(
                                ps[half * 4 + tb][:],
                                wsl,
                                xt_sb[:, kt * S + tb * 512:kt * S + (tb + 1) * 512],
                                start=(kt == 0), stop=(kt == KT - 1),
                                skip_group_check=True,
                            )
                # evac split across scalar+vector for faster bank turnover
                for i in range(8):
                    half, tb = divmod(i, 4)
                    dest = (qt if half == 0 else ktt)[h]
                    col = h if half == 0 else 4 + h
                    dsl = dest[:, tb * 512:(tb + 1) * 512]
                    if i % 2 == 0:
                        nc.scalar.add(dsl, ps[i][:], bqk_sb[:, col:col + 1])
                    else:
                        nc.vector.tensor_scalar_add(dsl, ps[i][:],
                                                    bqk_sb[:, col:col + 1])

        # ---- phase 2: attention + c_proj, pair-granularity weave ----
        with (
            tc.tile_pool(name="p2", bufs=1) as p2,
            tc.tile_pool(name="ps2a", bufs=1, space="PSUM") as ps2a,
        ):
            e_store = {}
            rcp_store = {}
            cnt = [0]

            def s_pairs(qb, h):
                es = []
                e_store[(qb, h)] = es

                def mk(p):
                    def go():
                        pss = ps2a.tile([128, 1024], F32, tag="pss", bufs=2,
                                        name="pss")
                        for half in range(2):
                            kt = 2 * p + half
                            nc.tensor.matmul(
                                pss[:, half * 512:(half + 1) * 512],
                                ktt[h][:, kt * 128:(kt + 1) * 128],
                                qt[h][:, qb * 512:(qb + 1) * 512],
                                start=True, stop=True,
                            )
                        e = p2.tile([128, 1024], F16, tag=f"e{p}", bufs=3,
                                    name=f"e{p}")
                        nc.scalar.activation(
                            e[:], pss[:], mybir.ActivationFunctionType.Exp,
                            scale=SCALE,
                        )
                        if use_mask:
                            for half in range(2):
                                kt = 2 * p + half
                                sl = e[:, half * 512:(half + 1) * 512]
                                nc.vector.tensor_scalar_mul(
                                    sl, sl, kb_sb[:, kt:kt + 1])
                        es.append(e)
                    return go

                def tail():
                    # denominator tree on DVE; partition all-reduce on
                    # GpSimd. The reciprocal is deferred to the AV block one
                    # slot later so the all-reduce latency never stalls the
                    # in-order DVE queue.
                    l1s = []
                    for i in range(4):
                        l1 = p2.tile([128, 1024], F16, tag=f"l1_{i % 2}",
                                     bufs=2, name=f"l1_{i % 2}")
                        nc.vector.tensor_add(l1[:], es[2 * i][:],
                                             es[2 * i + 1][:])
                        l1s.append(l1)
                    l2s = []
                    for j in range(2):
                        l2 = p2.tile([128, 1024], F16, tag=f"l2_{j}", bufs=2,
                                     name=f"l2_{j}")
                        nc.vector.tensor_add(l2[:], l1s[2 * j][:],
                                             l1s[2 * j + 1][:])
                        l2s.append(l2)
                    l3 = p2.tile([128, 1024], F16, tag="l3", bufs=2, name="l3")
                    nc.vector.tensor_add(l3[:], l2s[0][:], l2s[1][:])
                    dn = p2.tile([128, 512], F32, tag="dn", bufs=2, name="dn")
                    nc.vector.tensor_add(dn[:], l3[:, 0:512], l3[:, 512:1024])
                    dnr = p2.tile([128, 512], F32, tag="dnr", bufs=2,
                                  name="dnr")
                    nc.gpsimd.partition_all_reduce(
                        dnr[:], dn[:], channels=128,
                        reduce_op=bass_isa.ReduceOp.add)
                    rcp_store[(qb, h)] = dnr

                return [mk(p) for p in range(8)], tail

            def a_chunks(qb, h):
                psot_box = [None]

                def mk(c):
                    def go():
                        if c == 0:
                            psot_box[0] = ps2a.tile([128, 512], F32,
                                                    tag="psot", bufs=2,
                                                    name="psot")
                        es = e_store[(qb, h)]
                        for kt in range(4 * c, 4 * c + 4):
                            nc.tensor.matmul(
                                psot_box[0][:],
                                vaug[kt][h][:],
                                es[kt // 2][:, (kt % 2) * 512:(kt % 2 + 1) * 512],
                                start=(kt == 0), stop=(kt == KT - 1),
                            )
                        if c == 3:
                            rcp = p2.tile([128, 512], F32, tag="rcp", bufs=2,
                                          name="rcp")
                            nc.vector.reciprocal_approx_fast(
                                rcp[:], rcp_store.pop((qb, h))[:])
                            nc.vector.tensor_mul(
                                ot[h][:, qb * 512:(qb + 1) * 512],
                                psot_box[0][:], rcp[:])
                            e_store.pop((qb, h))
                    return go

                return [mk(c) for c in range(4)]

            stage_box = [None]

            def c_units(t, pool):
                def mk(nb):
                    def go():
                        if nb == 0:
                            stage_box[0] = p2.tile([128, S], F16, tag="stage",
                                                   bufs=3, name="stage")
                        psp = pool.tile([128, 512], F32,
                                        tag=f"psp{cnt[0] % 2}", bufs=1,
                                        name=f"psp{cnt[0] % 2}")
                        cnt[0] += 1
                        for h in range(HL):
                            nc.tensor.matmul(
                                psp[:],
                                ot[h][:, t * 128:(t + 1) * 128],
                                wp_sb[:, h * D + nb * 512:h * D + (nb + 1) * 512],
                                start=(h == 0), stop=(h == HL - 1),
                                skip_group_check=True,
                            )
                        st = stage_box[0]
                        nc.vector.tensor_copy(st[:, nb * 512:(nb + 1) * 512],
                                              psp[:])
                        if nb == 3:
                            eng = nc.sync if t % 2 else nc.scalar
                            eng.dma_start(out_ap[t * 128:(t + 1) * 128, :],
                                          st[:])
                    return go

                return [mk(nb) for nb in range(4)]

            def weave(pairs_tail, units):
                pairs, tail = pairs_tail
                for i in range(8):
                    pairs[i]()
                    if i < len(units):
                        units[i]()
                for u in units[8:]:
                    u()
                tail()

            # leftover V tiles ride the psot tag (same shape/banks, first AV
            # comes after the last of these).
            weave(s_pairs(0, 0),
                  v_chunks(10, ps2a, "psot", 2) + v_chunks(11, ps2a, "psot", 2))
            weave(s_pairs(0, 1),
                  v_chunks(12, ps2a, "psot", 2) + v_chunks(13, ps2a, "psot", 2))
            weave(s_pairs(0, 2),
                  v_chunks(14, ps2a, "psot", 2) + v_chunks(15, ps2a, "psot", 2))
            weave(s_pairs(0, 3), a_chunks(0, 0) + a_chunks(0, 1))

            with tc.tile_pool(name="ps2c", bufs=1, space="PSUM") as ps2c:
                weave(s_pairs(1, 0), a_chunks(0, 2) + a_chunks(0, 3))
                # steady slots: S(qb,h) ⊗ [A(prev head), c_proj tile]
                slots = [(qb, h) for qb in range(1, QB) for h in range(HL)][1:]
                for m, (qb, h) in enumerate(slots):
                    prev = (qb, h - 1) if h > 0 else (qb - 1, 3)
                    units = a_chunks(*prev) + c_units(m, ps2c)
                    if (qb, h) == (QB - 1, HL - 1):
                        units += c_units(m + 1, ps2c)
                    weave(s_pairs(qb, h), units)
                for u in a_chunks(QB - 1, 3):
                    u()
                for t in range(12, TT):
                    for u in c_units(t, ps2c):
                        u()

    nc.compile()
    nc.m = get_hw_module(nc.m)
    return nc


def kernel(hidden_states, attention_mask, w_attn, b_attn, w_proj, b_proj):
    global LAST_RESULTS
    hidden_states = np.asarray(hidden_states, dtype=np.float32)
    attention_mask = np.asarray(attention_mask, dtype=np.float32)
    w_attn = np.asarray(w_attn, dtype=np.float32)
    b_attn = np.asarray(b_attn, dtype=np.float32)
    w_proj = np.asarray(w_proj, dtype=np.float32)
    b_proj = np.asarray(b_proj, dtype=np.float32)

    use_mask = bool((attention_mask != 1.0).any())
    key = ("prog", use_mask)
    if key not in _PROG_CACHE:
        _PROG_CACHE[key] = _build_program(use_mask)
    nc = _PROG_CACHE[key]

    in_maps = []
    for c in range(N_CORES):
        b, g = divmod(c, 4)
        X = np.ascontiguousarray(hidden_states[b].T).astype(NPF16)  # [D, S]
        xt = np.ascontiguousarray(
            X.reshape(KT, 128, S).transpose(1, 0, 2).reshape(128, KT * S))
        xv = np.ascontiguousarray(
            X.reshape(KT, 128, TT, 128).transpose(1, 2, 0, 3)
            .reshape(128, TT * S))
        wq = w_attn[:, g * FL:(g + 1) * FL]
        wk = w_attn[:, D + g * FL:D + (g + 1) * FL]
        wvl = w_attn[:, 2 * D + g * FL:2 * D + (g + 1) * FL]
        A = wq.reshape(KT, 128, HL, 128)
        Bm = wk.reshape(KT, 128, HL, 128)
        wqk = np.ascontiguousarray(
            np.stack([A, Bm], axis=3).transpose(1, 2, 0, 3, 4)
            .reshape(128, HL * 4096)).astype(NPF16)
        wv = np.ascontiguousarray(
            wvl.reshape(KT, 128, FL).transpose(1, 0, 2)
            .reshape(128, KT * FL)).astype(NPF16)
        wp = np.ascontiguousarray(
            w_proj[g * FL:(g + 1) * FL, :].reshape(HL, 128, D)
            .transpose(1, 0, 2).reshape(128, HL * D)).astype(NPF16)
        bq = b_attn[g * FL:(g + 1) * FL]
        bk = b_attn[D + g * FL:D + (g + 1) * FL]
        bqk = np.ascontiguousarray(
            np.concatenate([bq, bk]).reshape(8, 128).T).astype(np.float32)
        kb = np.ascontiguousarray(
            attention_mask[b].reshape(KT, 128).T).astype(np.float32)
        in_maps.append({
            "xt": xt,
            "xv": xv,
            "wqk": wqk,
            "wv": wv,
            "wp": wp,
            "bqk": bqk,
            "kb": kb,
        })

    if not os.environ.get("KERNEL_ALLOW_TRACE"):
        os.environ["BASS_NEVER_TRACE"] = "1"
    try:
        res = bass_utils.run_bass_kernel_spmd(nc, in_maps,
                                              list(range(N_CORES)))
    except Exception:
        # Transient NRT failures can leave the axon device wedged; reset it
        # once and retry. If the reset path is unavailable, the retry's own
        # failure propagates.
        try:
            import ctypes

            import jax

            jax.devices()
            _lib = ctypes.CDLL("/opt/axon/libaxon_pjrt.so")
            _lib.axon_reset.restype = ctypes.c_int64
            _lib.axon_reset()
        except Exception:
            pass
        res = bass_utils.run_bass_kernel_spmd(nc, in_maps,
                                              list(range(N_CORES)))
    LAST_RESULTS = res

    # host reduce: sum the 4 head-group partials per batch, add biases.
    # V-bias contribution: rows of A sum to 1, so each core's O gains b_v
    # per row; through c_proj that's a constant row b_v @ w_proj_slice.
    out = np.zeros((B, S, D), dtype=np.float32)
    for c in range(N_CORES):
        b, g = divmod(c, 4)
        out[b] += res.results[c]["out"].astype(np.float32)
    bias_row = b_proj.astype(np.float64).copy()
    for g in range(4):
        bv = b_attn[2 * D + g * FL:2 * D + (g + 1) * FL].astype(np.float64)
        bias_row += bv @ w_proj[g * FL:(g + 1) * FL, :].astype(np.float64)
    out += bias_row.astype(np.float32)[None, None, :]
    return out


# revision 8
# speedup vs baseline: 1.6290x; 1.0453x over previous
"""Decoder-only attention block (QKV proj + MHA + out proj) on 8 TRN2 cores.

Sharding: core c -> (batch b = c//4, head-group g = c%4). Tensor-parallel over
heads (4 of 16 heads per core), data-parallel over batch (2). Each core
computes a partial c_proj over its 512 input features; host reduces the 4
partials per batch and adds biases.

Schedule: V projection first (it needs only ~7 MB of input, so it hides the
bulk DMA stream, which is paced behind it by explicit deps), then Q/K
projection with 8 concurrent PSUM chains, then a pair-granularity weave of
score matmuls with AV / c_proj / leftover-V filler so the in-order Tensor
queue never blocks on the scalar-engine exp pipeline.

Self-contained: hardcodes B=2, S=2048, D=2048, H=16.
"""

import os

import numpy as np

NPF16 = np.float16

import concourse.bass as bass
import concourse.bacc as bacc
import concourse.tile as tile
from concourse import mybir
import concourse.bass_utils as bass_utils
import concourse.bass_isa as bass_isa
from concourse.bass_interp import get_hw_module
from concourse.tile import add_dep_helper

B, S, D = 2, 2048, 2048
H, DH = 16, 128
N_CORES = 8
HL = H // 4            # 4 heads per core
FL = HL * DH           # 512 local features per core
KT = D // 128          # 16 contraction tiles
TT = S // 128          # 16 token tiles
QB = S // 512          # 4 token blocks
SCALE = 1.0 / float(np.sqrt(DH))

F16 = mybir.dt.float16
F32 = mybir.dt.float32

# Stash of the last BassKernelResults (for the local test harness only).
LAST_RESULTS = None
_PROG_CACHE = {}


def _build_program(use_mask):

    nc = bacc.Bacc("TRN2", target_bir_lowering=False, debug=False,
                   num_devices=N_CORES)

    # kt-major x for QK (moving operand), t-major copy for V (stationary).
    xt_d = nc.dram_tensor("xt", [128, KT * S], F16, kind="ExternalInput")
    xv_d = nc.dram_tensor("xv", [128, TT * S], F16, kind="ExternalInput")
    # head-pair-major QK weights: [row, h*4096 + kt*256 + half*128 + col]
    wqk_d = nc.dram_tensor("wqk", [128, HL * 4096], F16, kind="ExternalInput")
    wv_d = nc.dram_tensor("wv", [128, KT * FL], F16, kind="ExternalInput")
    wp_d = nc.dram_tensor("wp", [128, HL * D], F16, kind="ExternalInput")
    bqk_d = nc.dram_tensor("bqk", [128, 8], F32, kind="ExternalInput")
    kb_d = nc.dram_tensor("kb", [128, KT], F32, kind="ExternalInput")
    out_d = nc.dram_tensor("out", [S, D], F16, kind="ExternalOutput")

    xt_ap, xv_ap, wqk_ap = xt_d.ap(), xv_d.ap(), wqk_d.ap()
    wv_ap, wp_ap = wv_d.ap(), wp_d.ap()
    bqk_ap, kb_ap, out_ap = bqk_d.ap(), kb_d.ap(), out_d.ap()

    with tile.TileContext(nc) as tc, tc.tile_pool(name="pers", bufs=1) as pers:
        # ---- persistent tiles (live across phases) ----
        qt = [pers.tile([128, S], F16, tag=f"qt{h}", name=f"qt{h}") for h in range(HL)]
        ktt = [pers.tile([128, S], F16, tag=f"kt{h}", name=f"ktt{h}") for h in range(HL)]
        ot = [pers.tile([128, S], F16, tag=f"ot{h}", name=f"ot{h}") for h in range(HL)]
        vaug = [[pers.tile([128, DH], F16, tag=f"v{t}_{h}", name=f"v{t}_{h}")
                 for h in range(HL)] for t in range(TT)]
        wp_sb = pers.tile([128, HL * D], F16, tag="wp", name="wp_sb")
        wv_sb = pers.tile([128, KT * FL], F16, tag="wv", name="wv_sb")
        bqk_sb = pers.tile([128, 8], F32, tag="bqk", name="bqk_sb")
        kb_sb = pers.tile([128, KT], F32, tag="kb", name="kb_sb")
        warm = pers.tile([128, 512], F16, tag="warm", name="warm")

        # small side transfers ride the SWDGE queue; bulk uses the two HWDGE
        # rings (sync + scalar).
        nc.gpsimd.dma_start(bqk_sb[:], bqk_ap[:])
        nc.gpsimd.dma_start(kb_sb[:], kb_ap[:])
        nc.gpsimd.memset(warm[:], 0.0)

        # rolling t-major x buffers for V, two tiles per DMA (1 MB transfers
        # amortize the per-transfer fixed cost). Persistent pool so DMA
        # writes never alias phase-local space.
        xv_box = [None]
        last_mm = [None]

        def v_chunks(t, pool, tag, bufs):
            box = [None]

            def mk(c):
                def go():
                    if c == 0:
                        if t % 2 == 0:
                            xvt = pers.tile([128, 2 * S], F16, tag="xv",
                                            bufs=2, name="xv")
                            nc.sync.dma_start(
                                xvt[:], xv_ap[:, t * S:(t + 2) * S])
                            xv_box[0] = xvt
                        box[0] = pool.tile([128, FL], F32, tag=tag,
                                           bufs=bufs, name=tag)
                    psv = box[0]
                    off = (t % 2) * S
                    for kt in range(4 * c, 4 * c + 4):
                        last_mm[0] = nc.tensor.matmul(
                            psv[:],
                            xv_box[0][:, off + kt * 128:off + (kt + 1) * 128],
                            wv_sb[:, kt * FL:(kt + 1) * FL],
                            start=(kt == 0), stop=(kt == KT - 1),
                        )
                    if c == 3:
                        for h in range(HL):
                            nc.vector.tensor_copy(vaug[t][h][:],
                                                  psv[:, h * 128:(h + 1) * 128])
                return go

            return [mk(c) for c in range(4)]

        with (
            tc.tile_pool(name="pxt", bufs=1) as pxt,
            tc.tile_pool(name="pwqk", bufs=1) as pwqk,
        ):
            xt_sb = pxt.tile([128, KT * S], F16, tag="xt", name="xt_sb")

            # ---- phase 0: V tiles 0-9 cover the input DMA stream ----
            with tc.tile_pool(name="psv0", bufs=1, space="PSUM") as psv0:
                nc.sync.dma_start(wv_sb[:, 0:8 * FL], wv_ap[:, 0:8 * FL])
                # PE warm-up: HAM needs ~3.4us of sustained activity to lift
                # the clock gate; burn the initial DMA wait on dummy matmuls.
                wps = psv0.tile([128, FL], F32, tag="psv", bufs=2, name="wps")
                for _ in range(14):
                    nc.tensor.matmul(wps[:], warm[:, 0:128], warm[:],
                                     start=True, stop=True,
                                     skip_group_check=True)
                u0 = v_chunks(0, psv0, "psv", 2)
                u0[0]()
                nc.sync.dma_start(wv_sb[:, 8 * FL:KT * FL],
                                  wv_ap[:, 8 * FL:KT * FL])
                for u in u0[1:]:
                    u()
                for t in range(1, 10):
                    for u in v_chunks(t, psv0, "psv", 2):
                        u()
                    if t % 2 == 1 and t < 9:
                        # stream the QK-phase x behind V compute so it never
                        # steals bandwidth from the latency-critical xv/wv.
                        k0 = (t // 2) * 4
                        d = nc.scalar.dma_start(
                            xt_sb[:, k0 * S:(k0 + 4) * S],
                            xt_ap[:, k0 * S:(k0 + 4) * S])
                        add_dep_helper(d.ins, last_mm[0].ins, sync=False,
                                       reason="pace xt behind V")
            nc.scalar.dma_start(wp_sb[:], wp_ap[:])

            # ---- phase 1: Q/K projection, 8 concurrent PSUM chains ----
            with tc.tile_pool(name="psqk", bufs=1, space="PSUM") as psqk:
                for h in range(HL):
                    wqk_sb = pwqk.tile([128, 4096], F16, tag="wqk", bufs=2,
                                       name="wqk_sb")
                    nc.sync.dma_start(wqk_sb[:],
                                      wqk_ap[:, h * 4096:(h + 1) * 4096])
                    ps = [psqk.tile([128, 512], F32, tag=f"psqk{i}", bufs=1,
                                    name=f"psqk{i}") for i in range(8)]
                    for kt in range(KT):
                        for half in range(2):
                            base = kt * 256 + half * 128
                            wsl = wqk_sb[:, base:base + 128]
                            for tb in range(4):
                                nc.tensor.matmul(
                                    ps[half * 4 + tb][:],
                                    wsl,
                                    xt_sb[:, kt * S + tb * 512:kt * S + (tb + 1) * 512],
                                    start=(kt == 0), stop=(kt == KT - 1),
                                    skip_group_check=True,
                                )
                    # evac split across scalar+vector for faster bank turnover
                    for i in range(8):
                        half, tb = divmod(i, 4)
                        dest = (qt if half == 0 else ktt)[h]
                        col = h if half == 0 else 4 + h
                        dsl = dest[:, tb * 512:(tb + 1) * 512]
                        if i % 2 == 0:
                            nc.scalar.add(dsl, ps[i][:], bqk_sb[:, col:col + 1])
                        else:
                            nc.vector.tensor_scalar_add(dsl, ps[i][:],
                                                        bqk_sb[:, col:col + 1])

        # ---- phase 2: attention + c_proj, pair-granularity weave ----
        with (
            tc.tile_pool(name="p2", bufs=1) as p2,
            tc.tile_pool(name="ps2a", bufs=1, space="PSUM") as ps2a,
        ):
            e_store = {}
            rcp_store = {}
            cnt = [0]

            def s_pairs(qb, h):
                es = []
                e_store[(qb, h)] = es

                def mk(p):
                    def go():
                        pss = ps2a.tile([128, 1024], F32, tag="pss", bufs=2,
                                        name="pss")
                        for half in range(2):
                            kt = 2 * p + half
                            nc.tensor.matmul(
                                pss[:, half * 512:(half + 1) * 512],
                                ktt[h][:, kt * 128:(kt + 1) * 128],
                                qt[h][:, qb * 512:(qb + 1) * 512],
                                start=True, stop=True,
                            )
                        e = p2.tile([128, 1024], F16, tag=f"e{p}", bufs=3,
                                    name=f"e{p}")
                        nc.scalar.activation(
                            e[:], pss[:], mybir.ActivationFunctionType.Exp,
                            scale=SCALE,
                        )
                        if use_mask:
                            for half in range(2):
                                kt = 2 * p + half
                                sl = e[:, half * 512:(half + 1) * 512]
                                nc.vector.tensor_scalar_mul(
                                    sl, sl, kb_sb[:, kt:kt + 1])
                        es.append(e)
                    return go

                def tail():
                    # denominator tree on DVE; partition all-reduce on
                    # GpSimd. The reciprocal is deferred to the AV block one
                    # slot later so the all-reduce latency never stalls the
                    # in-order DVE queue.
                    l1s = []
                    for i in range(4):
                        l1 = p2.tile([128, 1024], F16, tag=f"l1_{i % 2}",
                                     bufs=2, name=f"l1_{i % 2}")
                        nc.vector.tensor_add(l1[:], es[2 * i][:],
                                             es[2 * i + 1][:])
                        l1s.append(l1)
                    l2s = []
                    for j in range(2):
                        l2 = p2.tile([128, 1024], F16, tag=f"l2_{j}", bufs=2,
                                     name=f"l2_{j}")
                        nc.vector.tensor_add(l2[:], l1s[2 * j][:],
                                             l1s[2 * j + 1][:])
                        l2s.append(l2)
                    l3 = p2.tile([128, 1024], F16, tag="l3", bufs=2, name="l3")
                    nc.vector.tensor_add(l3[:], l2s[0][:], l2s[1][:])
                    dn = p2.tile([128, 512], F32, tag="dn", bufs=2, name="dn")
                    nc.vector.tensor_add(dn[:], l3[:, 0:512], l3[:, 512:1024])
                    dnr = p2.tile([128, 512], F32, tag="dnr", bufs=2,
                                  name="dnr")
                    nc.gpsimd.partition_all_reduce(
                        dnr[:], dn[:], channels=128,
                        reduce_op=bass_isa.ReduceOp.add)
                    rcp_store[(qb, h)] = dnr

                return [mk(p) for p in range(8)], tail

            def a_chunks(qb, h):
                psot_box = [None]

                def mk(c):
                    def go():
                        if c == 0:
                            psot_box[0] = ps2a.tile([128, 512], F32,
                                                    tag="psot", bufs=2,
                                                    name="psot")
                        es = e_store[(qb, h)]
                        for kt in range(4 * c, 4 * c + 4):
                            nc.tensor.matmul(
                                psot_box[0][:],
                                vaug[kt][h][:],
                                es[kt // 2][:, (kt % 2) * 512:(kt % 2 + 1) * 512],
                                start=(kt == 0), stop=(kt == KT - 1),
                            )
                        if c == 3:
                            rcp = p2.tile([128, 512], F32, tag="rcp", bufs=2,
                                          name="rcp")
                            nc.vector.reciprocal_approx_fast(
                                rcp[:], rcp_store.pop((qb, h))[:])
                            nc.vector.tensor_mul(
                                ot[h][:, qb * 512:(qb + 1) * 512],
                                psot_box[0][:], rcp[:])
                            e_store.pop((qb, h))
                    return go

                return [mk(c) for c in range(4)]

            stage_box = [None]

            def c_units(t, pool):
                def mk(nb):
                    def go():
                        if nb == 0:
                            stage_box[0] = p2.tile([128, S], F16, tag="stage",
                                                   bufs=3, name="stage")
                        psp = pool.tile([128, 512], F32,
                                        tag=f"psp{cnt[0] % 2}", bufs=1,
                                        name=f"psp{cnt[0] % 2}")
                        cnt[0] += 1
                        for h in range(HL):
                            nc.tensor.matmul(
                                psp[:],
                                ot[h][:, t * 128:(t + 1) * 128],
                                wp_sb[:, h * D + nb * 512:h * D + (nb + 1) * 512],
                                start=(h == 0), stop=(h == HL - 1),
                                skip_group_check=True,
                            )
                        st = stage_box[0]
                        # split evacuation between ACT and DVE so neither
                        # queue backs up into the c_proj matmuls
                        if nb % 2 == 0:
                            nc.scalar.copy(st[:, nb * 512:(nb + 1) * 512],
                                           psp[:])
                        else:
                            nc.vector.tensor_copy(
                                st[:, nb * 512:(nb + 1) * 512], psp[:])
                        if nb == 3:
                            eng = nc.sync if t % 2 else nc.scalar
                            eng.dma_start(out_ap[t * 128:(t + 1) * 128, :],
                                          st[:])
                    return go

                return [mk(nb) for nb in range(4)]

            def weave(pairs_tail, units):
                pairs, tail = pairs_tail
                for i in range(8):
                    pairs[i]()
                    if i < len(units):
                        units[i]()
                for u in units[8:]:
                    u()
                tail()

            # leftover V tiles ride the psot tag (same shape/banks, first AV
            # comes after the last of these).
            weave(s_pairs(0, 0),
                  v_chunks(10, ps2a, "psot", 2) + v_chunks(11, ps2a, "psot", 2))
            weave(s_pairs(0, 1),
                  v_chunks(12, ps2a, "psot", 2) + v_chunks(13, ps2a, "psot", 2))
            weave(s_pairs(0, 2),
                  v_chunks(14, ps2a, "psot", 2) + v_chunks(15, ps2a, "psot", 2))
            weave(s_pairs(0, 3), a_chunks(0, 0) + a_chunks(0, 1))

            with tc.tile_pool(name="ps2c", bufs=1, space="PSUM") as ps2c:
                weave(s_pairs(1, 0), a_chunks(0, 2) + a_chunks(0, 3))
                # steady slots: S(qb,h) ⊗ [A(prev head), c_proj tile]
                slots = [(qb, h) for qb in range(1, QB) for h in range(HL)][1:]
                for m, (qb, h) in enumerate(slots):
                    prev = (qb, h - 1) if h > 0 else (qb - 1, 3)
                    units = a_chunks(*prev) + c_units(m, ps2c)
                    if (qb, h) == (QB - 1, HL - 1):
                        units += c_units(m + 1, ps2c)
                    weave(s_pairs(qb, h), units)
                for u in a_chunks(QB - 1, 3):
                    u()
                for t in range(12, TT):
                    for u in c_units(t, ps2c):
                        u()

    nc.compile()
    nc.m = get_hw_module(nc.m)
    return nc


def kernel(hidden_states, attention_mask, w_attn, b_attn, w_proj, b_proj):
    global LAST_RESULTS
    hidden_states = np.asarray(hidden_states, dtype=np.float32)
    attention_mask = np.asarray(attention_mask, dtype=np.float32)
    w_attn = np.asarray(w_attn, dtype=np.float32)
    b_attn = np.asarray(b_attn, dtype=np.float32)
    w_proj = np.asarray(w_proj, dtype=np.float32)
    b_proj = np.asarray(b_proj, dtype=np.float32)

    use_mask = bool((attention_mask != 1.0).any())
    key = ("prog", use_mask)
    if key not in _PROG_CACHE:
        _PROG_CACHE[key] = _build_program(use_mask)
    nc = _PROG_CACHE[key]

    in_maps = []
    for c in range(N_CORES):
        b, g = divmod(c, 4)
        X = np.ascontiguousarray(hidden_states[b].T).astype(NPF16)  # [D, S]
        xt = np.ascontiguousarray(
            X.reshape(KT, 128, S).transpose(1, 0, 2).reshape(128, KT * S))
        xv = np.ascontiguousarray(
            X.reshape(KT, 128, TT, 128).transpose(1, 2, 0, 3)
            .reshape(128, TT * S))
        wq = w_attn[:, g * FL:(g + 1) * FL]
        wk = w_attn[:, D + g * FL:D + (g + 1) * FL]
        wvl = w_attn[:, 2 * D + g * FL:2 * D + (g + 1) * FL]
        A = wq.reshape(KT, 128, HL, 128)
        Bm = wk.reshape(KT, 128, HL, 128)
        wqk = np.ascontiguousarray(
            np.stack([A, Bm], axis=3).transpose(1, 2, 0, 3, 4)
            .reshape(128, HL * 4096)).astype(NPF16)
        wv = np.ascontiguousarray(
            wvl.reshape(KT, 128, FL).transpose(1, 0, 2)
            .reshape(128, KT * FL)).astype(NPF16)
        wp = np.ascontiguousarray(
            w_proj[g * FL:(g + 1) * FL, :].reshape(HL, 128, D)
            .transpose(1, 0, 2).reshape(128, HL * D)).astype(NPF16)
        bq = b_attn[g * FL:(g + 1) * FL]
        bk = b_attn[D + g * FL:D + (g + 1) * FL]
        bqk = np.ascontiguousarray(
            np.concatenate([bq, bk]).reshape(8, 128).T).astype(np.float32)
        kb = np.ascontiguousarray(
            attention_mask[b].reshape(KT, 128).T).astype(np.float32)
        in_maps.append({
            "xt": xt,
            "xv": xv,
            "wqk": wqk,
            "wv": wv,
            "wp": wp,
            "bqk": bqk,
            "kb": kb,
        })

    if not os.environ.get("KERNEL_ALLOW_TRACE"):
        os.environ["BASS_NEVER_TRACE"] = "1"
    try:
        res = bass_utils.run_bass_kernel_spmd(nc, in_maps,
                                              list(range(N_CORES)))
    except Exception:
        # Transient NRT failures can leave the axon device wedged; reset it
        # once and retry. If the reset path is unavailable, the retry's own
        # failure propagates.
        try:
            import ctypes

            import jax

            jax.devices()
            _lib = ctypes.CDLL("/opt/axon/libaxon_pjrt.so")
            _lib.axon_reset.restype = ctypes.c_int64
            _lib.axon_reset()
        except Exception:
            pass
        res = bass_utils.run_bass_kernel_spmd(nc, in_maps,
                                              list(range(N_CORES)))
    LAST_RESULTS = res

    # host reduce: sum the 4 head-group partials per batch, add biases.
    # V-bias contribution: rows of A sum to 1, so each core's O gains b_v
    # per row; through c_proj that's a constant row b_v @ w_proj_slice.
    out = np.zeros((B, S, D), dtype=np.float32)
    for c in range(N_CORES):
        b, g = divmod(c, 4)
        out[b] += res.results[c]["out"].astype(np.float32)
    bias_row = b_proj.astype(np.float64).copy()
    for g in range(4):
        bv = b_attn[2 * D + g * FL:2 * D + (g + 1) * FL].astype(np.float64)
        bias_row += bv @ w_proj[g * FL:(g + 1) * FL, :].astype(np.float64)
    out += bias_row.astype(np.float32)[None, None, :]
    return out
